# revision 25
# baseline (speedup 1.0000x reference)
"""Causal self-attention (B=2, T=2048, D=1024, H=16, rope) on 8 Trainium2 cores.

Sharding: heads are split across cores (2 heads/core, tensor-parallel):
each core computes QKV projection columns for its heads, RoPE, causal
attention, and a partial out-projection (its rows of w_out). The host sums
the 8 partial outputs (the tensor-parallel all-reduce, done at gather time).

All matmul operands are fp16 (fp32 PSUM accumulation). Activations flow
feature-major (transposed); the host transposes x on the way in and the
output back on the way out. Softmax denominators come free from a
ones-column appended to V; exp runs biased (exp(s/8 - 4)) to stay inside
fp16 range, the bias cancels in the normalization.

v2 layout/scheduling notes:
- q/k feature rows are host-permuted so rope's rotate-half partner always
  sits 16 rows away inside the same 32-partition quadrant: the rotate
  becomes one DVE stream_shuffle instead of 4 SBUF-SBUF DMAs. Scores are
  invariant to any per-head feature permutation applied to both q and k.
- The two heads' score matmuls (64-row contractions) are packed into the
  128x128 PE array concurrently via tile_position row groups.
- Score/AV matmuls skip fully-masked leading columns of diagonal blocks;
  only the remaining 128-wide triangle gets a mask multiply (post-exp).
- Out-projection is interleaved per 512-token q-slice with attention, and
  batch 1's QKV projection overlaps batch 0's attention.
- Partial outputs are stored fp16, one batched DMA per (batch, j-slice).
"""

import sys

for _p in ("/opt/trn_rl_repo",):
    if _p not in sys.path:
        sys.path.insert(0, _p)

import numpy as np

B, T, D, H = 2, 2048, 1024, 16
DH = D // H  # 64
N_CORES = 8
HPC = H // N_CORES  # heads per core = 2
BT = B * T  # 4096
ROPE_BASE = 10000.0
EXP_BIAS = -4.0

NK = D // 128       # 8 contraction chunks for qkv projection
NS = BT // 512      # 8 token slices
NJ = T // 512       # 4 tq slices per batch
NB = T // 128       # 16 tk blocks per batch

# rope feature permutation within one 64-row head: rows [0:32] hold
# features [0..15, 32..47], rows [32:64] hold [16..31, 48..63] — the
# rotate-half partner of row r is always (r+16)%32 within r's quadrant.
PERM64 = (list(range(0, 16)) + list(range(32, 48))
          + list(range(16, 32)) + list(range(48, 64)))
SHUF_MASK = [(i + 16) % 32 for i in range(32)]

_CACHE = {}


def _host_consts():
    inv_freq = 1.0 / (ROPE_BASE ** (np.arange(0, DH, 2, dtype=np.float32) / DH))
    t = np.arange(T, dtype=np.float32)
    # per permuted row r: feature f = PERM64[r], freq index f%32,
    # sin sign - for f<32 (gets -x2 partner), + for f>=32
    cos64 = np.empty((64, T), dtype=np.float32)
    sin64 = np.empty((64, T), dtype=np.float32)
    for r, f in enumerate(PERM64):
        ang = t * inv_freq[f % 32]
        cos64[r] = np.cos(ang)
        sin64[r] = np.sin(ang) * (-1.0 if f < 32 else 1.0)
    cosb = np.concatenate([cos64, cos64], axis=0).astype(np.float16)
    sinb = np.concatenate([sin64, sin64], axis=0).astype(np.float16)
    # upper-triangle keep-mask for the 128-wide partially-valid column
    # band of each diagonal block: tri[p, c] = 1 if c >= p
    p = np.arange(128)
    tri = (p[None, :] >= p[:, None]).astype(np.float16)
    return cosb, sinb, tri


def _build(debug=False, reps=1, parts="full"):
    """Build + schedule the per-core Bass module (same program on all cores).
    reps>1 repeats the whole body (timing diagnostic: slope between reps
    NEFFs isolates pure body time). parts: timing-bisect variants — "full",
    "noout" (skip out-projection), "noav" (skip AV/norm/outproj too),
    "noatt" (qkv+rope+vprep only). Non-full variants produce garbage
    outputs; timing only."""
    from concourse import bacc
    import concourse.mybir as mybir
    import concourse.tile as tile

    F16 = mybir.dt.float16
    F32 = mybir.dt.float32
    AF = mybir.ActivationFunctionType

    nc = bacc.Bacc("TRN2", target_bir_lowering=False, debug=False,
                   num_devices=N_CORES)

    xt_d = nc.dram_tensor("xt", [D, BT], F16, kind="ExternalInput")
    # weights host-prearranged to [128 partitions, k*128] so the load is
    # contiguous per partition (2KB chunks = full DMA bandwidth)
    wq_d = nc.dram_tensor("wq", [128, D], F16, kind="ExternalInput")
    wk_d = nc.dram_tensor("wk", [128, D], F16, kind="ExternalInput")
    wv_d = nc.dram_tensor("wv", [128, D], F16, kind="ExternalInput")
    wo_d = nc.dram_tensor("wo", [128, D], F16, kind="ExternalInput")
    cos_d = nc.dram_tensor("cosb", [128, T], F16, kind="ExternalInput")
    sin_d = nc.dram_tensor("sinb", [128, T], F16, kind="ExternalInput")
    tri_d = nc.dram_tensor("tri", [128, 128], F16, kind="ExternalInput")
    out_d = nc.dram_tensor("outp", [D, BT], F16, kind="ExternalOutput")
    out_r = out_d[:, :].rearrange("(m p) t -> p m t", p=128)

    with tile.TileContext(nc) as tc:
        with (
            tc.tile_pool(name="consts", bufs=1) as consts,
            tc.tile_pool(name="acts", bufs=1) as acts,
            tc.tile_pool(name="on", bufs=2) as onp,
            tc.tile_pool(name="xt", bufs=3) as xtp,
            tc.tile_pool(name="rope", bufs=4) as rope,
            tc.tile_pool(name="vp", bufs=2) as vpp,
            tc.tile_pool(name="est", bufs=10) as estp,
            tc.tile_pool(name="inv", bufs=2) as invp,
            tc.tile_pool(name="oev", bufs=2) as oevp,
            tc.tile_pool(name="pp_ps", bufs=2, space="PSUM") as pp_ps,
            tc.tile_pool(name="st_ps", bufs=2, space="PSUM") as st_ps,
            tc.tile_pool(name="u_ps", bufs=1, space="PSUM") as u_ps,
        ):
            wq = consts.tile([128, NK, 128], F16)
            wk = consts.tile([128, NK, 128], F16)
            wv = consts.tile([128, NK, 128], F16)
            wo = consts.tile([128, NK, 128], F16)
            cosb = consts.tile([128, T], F16)
            sinb = consts.tile([128, T], F16)
            tri = consts.tile([128, 128], F16)
            # qkv weights first so the first projection matmul can start
            # as early as possible; cos/sin next (first rope), wo/tri later
            nc.sync.dma_start(out=wq, in_=wq_d[:, :].rearrange("p (k f) -> p k f", k=NK))
            ones16 = consts.tile([128, NB], F16)
            nc.vector.memset(ones16, 1.0)
            ebias = consts.tile([128, 1], F32)
            nc.vector.memset(ebias, EXP_BIAS)
            # dummy matmuls fill the initial input-DMA window so the PE's
            # HAM clock-gate is warm (2.4GHz) when the real stream starts
            warm = consts.tile([128, 512], F16)
            nc.vector.memset(warm, 0.0)
            wmp = pp_ps.tile([128, 512], F32, tag="pp", name="warm_ps")
            for _ in range(8):
                nc.tensor.matmul(wmp, warm[:, 0:128], warm,
                                 start=True, stop=True)

            qt = acts.tile([128, BT], F16)  # rows: [h0 d0..63 | h1 d0..63]
            kt = acts.tile([128, BT], F16)
            vt = acts.tile([128, BT], F16)

            xt_r = xt_d[:, :].rearrange("(k p) t -> p k t", p=128)

            rep = 0

            def qkv_load(n):
                xtt = xtp.tile([128, NK, 512], F16, tag="xt",
                               name=f"xtt{n}_r{rep}")
                nc.sync.dma_start(out=xtt, in_=xt_r[:, :, n * 512:(n + 1) * 512])
                return xtt

            def qkv_slice(n, xtt=None):
                """QKV^T projection + rope for 512-token slice n. xtt may be
                a single [128,NK,512] tile or two [128,NK/2,512] halves.
                PSUM evacuations run on ACT for slices 0-3 (ACT idle there)
                and on DVE for slices 4-7 (ACT is busy with exp then)."""
                ts = slice(n * 512, (n + 1) * 512)
                cs = slice((n % NJ) * 512, (n % NJ) * 512 + 512)
                on_act = n < 4
                if xtt is None:
                    xtt = qkv_load(n)
                halves = xtt if isinstance(xtt, (list, tuple)) else (xtt,)
                kper = NK // len(halves)
                for w, dst in ((wq, qt), (wk, kt), (wv, vt)):
                    ps = pp_ps.tile([128, 512], F32, tag="pp")
                    for k in range(NK):
                        nc.tensor.matmul(ps, w[:, k, :],
                                         halves[k // kper][:, k % kper, :],
                                         start=(k == 0), stop=(k == NK - 1))
                    if dst is vt:
                        nc.vector.tensor_copy(vt[:, ts], ps)
                    else:
                        raw = rope.tile([128, 512], F16, tag="raw")
                        if on_act:
                            nc.scalar.copy(raw, ps)
                        else:
                            nc.vector.tensor_copy(raw, ps)
                        swp = rope.tile([128, 512], F16, tag="swp")
                        nc.vector.stream_shuffle(swp, raw, SHUF_MASK)
                        nc.vector.tensor_mul(raw, raw, cosb[:, cs])
                        nc.vector.tensor_mul(swp, swp, sinb[:, cs])
                        nc.vector.tensor_add(dst[:, ts], raw, swp)

            def vprep_alloc(b):
                """V' = [V_h | 1] token-major; inner stride 80 elems = 160B
                keeps every transpose dst 32B-aligned (unaligned dsts
                corrupt silently). The constant ones column is written once
                per allocation (early, no deps) rather than per slice."""
                vph = [vpp.tile([128, NB, 80], F16, tag=f"vp{h}",
                                name=f"vp{h}_{b}_r{rep}")
                       for h in range(HPC)]
                for h in range(HPC):
                    nc.vector.tensor_copy(vph[h][:, :, 64], ones16[:, 0:NB])
                return vph

            def vprep_slice(b, vph, s):
                """Transpose one 512-token slice (4 blocks) of V per head."""
                t0 = b * T
                for h in range(HPC):
                    nc.sync.dma_start_transpose(
                        out=vph[h][:, 4 * s:4 * s + 4, 0:64],
                        in_=vt[h * 64:(h + 1) * 64,
                               t0 + s * 512:t0 + (s + 1) * 512])

            def att_jslice(b, j, vph, on, deferred):
                """Attention for q tokens [512j, 512j+512) of batch b, both
                heads. The previous j-slice's out-projection is spread
                across this slice's score groups (hides pp-bank evac
                latency); each head is normalized right after its final AV
                so the norm chain hides behind the other head's AVs."""
                t0 = b * T
                qs0 = t0 + j * 512
                nblk = 4 * j + 4
                ngrp = nblk // 2
                us = [u_ps.tile([65, 512], F32, tag=f"u{h}",
                                name=f"u{h}_{b}_{j}_r{rep}") for h in range(HPC)]

                def norm_head(h):
                    # r = u[64] (denominators); on = u[0:64] / r. u's two
                    # reads (numerator copy + denominator row) come first so
                    # the bank frees early; recip input must sit at
                    # base-partition 0 (custom-DVE op).
                    hp = h * 64
                    rrow = invp.tile([1, 512], F32, tag="rrow")
                    nc.vector.tensor_copy(rrow, us[h][64:65, :])
                    ucop = invp.tile([64, 512], F32, tag="ucop")
                    nc.vector.tensor_copy(ucop, us[h][0:64, :])
                    rinv = invp.tile([1, 512], F32, tag="rinv")
                    nc.vector.reciprocal_approx_fast(rinv, rrow)
                    bci = invp.tile([64, 512], F32, tag="bci")
                    nc.gpsimd.partition_broadcast(bci, rinv)
                    nc.gpsimd.tensor_mul(
                        on[hp:hp + 64, j * 512:(j + 1) * 512],
                        ucop, bci)

                def make_av(h, est, blks):
                    # AV step as a closure on the rep-wide avq pipeline:
                    # entries survive j-slice boundaries, so the next
                    # slice's scores hide this slice's est-chain latency
                    def run():
                        for t2, i, o in blks:
                            nc.tensor.matmul(
                                us[h][:, o:512], vph[h][:, i, 0:65],
                                est[:, t2 * 512 + o:(t2 + 1) * 512],
                                start=(i == 0), stop=(i == nblk - 1))
                        if blks[-1][1] == nblk - 1:
                            norm_head(h)
                    return run

                def run_deferred(g):
                    # run outproj steps two j-slices behind their own slice:
                    # only entries older than the newest NK run here, so the
                    # normalize chain producing `on` has a full j-slice of
                    # slack before the first outproj matmul reads it
                    avail = len(deferred) - NK
                    if avail > 0:
                        for _ in range(-(-avail // (ngrp - g))):
                            deferred.pop(0)()

                for g in range(ngrp):
                    for h in range(HPC):
                        hp = h * 64
                        st = st_ps.tile([128, 1024], F32, tag="st")
                        blks = []
                        for t2 in range(2):
                            i = 2 * g + t2
                            o = max(0, 128 * (i - 4 * j))
                            blks.append((t2, i, o))
                            nc.tensor.matmul(
                                st[:, t2 * 512 + o:(t2 + 1) * 512],
                                kt[hp:hp + 64,
                                   t0 + i * 128: t0 + (i + 1) * 128],
                                qt[hp:hp + 64, qs0 + o: qs0 + 512],
                                start=True, stop=True,
                                tile_position=(hp, 0))
                        est = estp.tile([128, 1024], F16, tag="est")
                        if blks[0][2] == 0 and blks[1][2] == 0:
                            nc.scalar.activation(est, st, AF.Exp,
                                                 scale=float(DH) ** -0.5,
                                                 bias=ebias)
                        else:  # diagonal group: exp only the written bands
                            for t2, i, o in blks:
                                c0, c1 = t2 * 512 + o, (t2 + 1) * 512
                                nc.scalar.activation(est[:, c0:c1],
                                                     st[:, c0:c1], AF.Exp,
                                                     scale=float(DH) ** -0.5,
                                                     bias=ebias)
                        for t2, i, o in blks:
                            if i >= 4 * j:  # diagonal: mask the 128-wide band
                                c0 = t2 * 512 + o
                                nc.gpsimd.tensor_mul(
                                    est[:, c0:c0 + 128], est[:, c0:c0 + 128],
                                    tri)
                        if parts not in ("noav",):
                            avq.append(make_av(h, est, blks))
                        if len(avq) > 3:
                            avq.pop(0)()
                    run_deferred(g)  # spread 2-slice-old outproj over groups
                if b == B - 1 and j == NJ - 1:
                    while avq:  # final drain, then keep PE warm through the
                        avq.pop(0)()  # last normalize + out-projection
                    wst = st_ps.tile([128, 1024], F32, tag="st",
                                     name=f"warm_tail_r{rep}")
                    for _ in range(12):
                        nc.tensor.matmul(wst[:, 0:512], warm[:, 0:128], warm,
                                         start=True, stop=True)

                # out-projection for this j-slice, as per-m steps the next
                # j-slice interleaves with its score groups; one batched
                # store DMA at the end (split in two for the final slice)
                ons = on[:, j * 512:(j + 1) * 512]
                ot = oevp.tile([128, NK, 512], F16, tag="ot",
                               name=f"ot_{b}_{j}_r{rep}")
                split = (b == B - 1 and j == NJ - 1)

                def outproj_step(m):
                    def run():
                        op = pp_ps.tile([128, 512], F32, tag="pp",
                                        name=f"op_{b}_{j}_{m}_r{rep}")
                        nc.tensor.matmul(op, wo[:, m, :], ons,
                                         start=True, stop=True)
                        nc.vector.tensor_copy(ot[:, m, :], op)
                        if split and m == NK // 2 - 1:
                            nc.sync.dma_start(
                                out=out_r[:, 0:NK // 2,
                                          t0 + j * 512: t0 + (j + 1) * 512],
                                in_=ot[:, 0:NK // 2, :])
                        if m == NK - 1:
                            nc.sync.dma_start(
                                out=out_r[:, NK // 2 if split else 0: NK,
                                          t0 + j * 512: t0 + (j + 1) * 512],
                                in_=ot[:, NK // 2 if split else 0: NK, :])
                    return run
                if parts == "full":
                    deferred.extend(outproj_step(m) for m in range(NK))

            # ---- pipelined emission: qkv(b0) | att(b0) x qkv(b1) | att(b1)
            # startup: first x-slice load goes on the DMA queue right after
            # wq, split in two so the first matmuls start after half loads;
            # wk/wv/cos/sin/wo/tri follow behind it
            xt0a = xtp.tile([128, NK // 2, 512], F16, tag="x0a", name="xt0a")
            nc.sync.dma_start(out=xt0a, in_=xt_r[:, 0:NK // 2, 0:512])
            xt0b = xtp.tile([128, NK // 2, 512], F16, tag="x0b", name="xt0b")
            nc.sync.dma_start(out=xt0b, in_=xt_r[:, NK // 2:NK, 0:512])
            xtt0 = (xt0a, xt0b)
            # remaining consts go on the Activation-engine DMA queue so the
            # SP queue carries only x loads + output stores (a const never
            # delays an x prefetch); const DMAs have no waits, so they can't
            # head-of-line block ACT's activations
            nc.scalar.dma_start(out=wk, in_=wk_d[:, :].rearrange("p (k f) -> p k f", k=NK))
            nc.scalar.dma_start(out=wv, in_=wv_d[:, :].rearrange("p (k f) -> p k f", k=NK))
            # slice 1 also split in halves so its first matmuls start early
            xt1a = xtp.tile([128, NK // 2, 512], F16, tag="x1a", name="xt1a")
            nc.sync.dma_start(out=xt1a, in_=xt_r[:, 0:NK // 2, 512:1024])
            xt1b = xtp.tile([128, NK // 2, 512], F16, tag="x1b", name="xt1b")
            nc.sync.dma_start(out=xt1b, in_=xt_r[:, NK // 2:NK, 512:1024])
            xtt1 = (xt1a, xt1b)
            nc.scalar.dma_start(out=cosb, in_=cos_d[:, :])
            nc.scalar.dma_start(out=sinb, in_=sin_d[:, :])
            nc.scalar.dma_start(out=wo, in_=wo_d[:, :].rearrange("p (m f) -> p m f", m=NK))
            nc.scalar.dma_start(out=tri, in_=tri_d[:, :])
            # preload the exp table set while ACT is otherwise idle so the
            # first real exp (on the attention critical path) doesn't pay
            # the ~2.7µs table-load
            wexp = consts.tile([128, 1], F32)
            nc.scalar.activation(wexp, ebias, AF.Exp)
            deferred = []
            avq = []  # cross-slice AV pipeline (see make_av)
            for rep in range(reps):
                if rep > 0:
                    xtt0 = qkv_load(0)
                    xtt1 = qkv_load(1)
                vph0 = vprep_alloc(0)
                qkv_slice(0, xtt0)
                vprep_slice(0, vph0, 0)
                xttn = qkv_load(2)  # prefetch one slice ahead
                qkv_slice(1, xtt1)
                vprep_slice(0, vph0, 1)
                for n in range(2, 4):
                    xtt, xttn = xttn, qkv_load(n + 1)
                    qkv_slice(n, xtt)
                    vprep_slice(0, vph0, n)
                vph1 = vprep_alloc(1)
                on0 = onp.tile([128, T], F16, tag="on", name=f"on0_r{rep}")
                on1 = onp.tile([128, T], F16, tag="on", name=f"on1_r{rep}")
                for j in range(NJ):
                    if parts != "noatt":
                        att_jslice(0, j, vph0, on0, deferred)
                    xtt = xttn
                    if j < NJ - 1:
                        xttn = qkv_load(5 + j)
                    qkv_slice(4 + j, xtt)
                    vprep_slice(1, vph1, j)
                if parts != "noatt":
                    for j in range(NJ):
                        att_jslice(1, j, vph1, on1, deferred)
            while deferred:
                deferred.pop(0)()

    nc.compile()
    return nc


def _get_nc(debug=False):
    key = "ncd" if debug else "nc"
    if key not in _CACHE:
        _CACHE[key] = _build(debug)
    return _CACHE[key]


def _run(nc, in_maps, trace=False):
    from concourse.bass_utils import run_bass_kernel_spmd

    last = None
    for attempt in range(3):
        try:
            return run_bass_kernel_spmd(nc, in_maps,
                                        core_ids=list(range(N_CORES)),
                                        trace=trace)
        except Exception as e:  # transient device faults: retry
            last = e
            if "UNRECOVERABLE" not in str(e) and "UNAVAILABLE" not in str(e):
                raise
    raise last


def _in_maps(x, w_qkv, w_out):
    x = np.asarray(x, dtype=np.float32)
    w_qkv = np.asarray(w_qkv, dtype=np.float32)
    w_out = np.asarray(w_out, dtype=np.float32)

    xt = np.ascontiguousarray(x.reshape(BT, D).T).astype(np.float16)
    cosb, sinb, tri = _host_consts()

    def wprep(w128):  # [D, 128] -> [128, k*128] partition-contiguous
        return np.ascontiguousarray(
            w128.reshape(NK, 128, 128).transpose(1, 0, 2).reshape(128, D)
        ).astype(np.float16)

    in_maps = []
    for c in range(N_CORES):
        h0 = HPC * c
        # q/k columns in rope-permuted order; v/wo in natural order
        pcols = np.concatenate(
            [np.array(PERM64) + (h0 + h) * DH for h in range(HPC)])
        cols = np.arange(h0 * DH, (h0 + HPC) * DH)
        in_maps.append({
            "xt": xt,
            "wq": wprep(w_qkv[:, pcols]),
            "wk": wprep(w_qkv[:, D + pcols]),
            "wv": wprep(w_qkv[:, 2 * D + cols]),
            "wo": np.ascontiguousarray(w_out[cols, :]).astype(np.float16),
            "cosb": cosb,
            "sinb": sinb,
            "tri": tri,
        })
    return in_maps


def kernel(x, w_qkv, w_out, _trace=False, _debug=False):
    in_maps = _in_maps(x, w_qkv, w_out)
    nc = _get_nc(_debug)
    res = _run(nc, in_maps, trace=_trace)
    acc = np.zeros((D, BT), dtype=np.float64)
    for c in range(N_CORES):
        acc += res.results[c]["outp"]
    out = acc.T.astype(np.float32).reshape(B, T, D)
    if _trace:
        return out, res
    return out



# revision 28
# speedup vs baseline: 3.2358x; 3.2358x over previous
"""Causal self-attention (B=2, T=2048, D=1024, H=16, rope) on 8 Trainium2 cores.

Sharding: heads are split across cores (2 heads/core, tensor-parallel):
each core computes QKV projection columns for its heads, RoPE, causal
attention, and a partial out-projection (its rows of w_out). The host sums
the 8 partial outputs (the tensor-parallel all-reduce, done at gather time).

All matmul operands are fp16 (fp32 PSUM accumulation). Activations flow
feature-major (transposed); the host transposes x on the way in and the
output back on the way out. Softmax denominators come free from a
ones-column appended to V; exp runs biased (exp(s/8 - 4)) to stay inside
fp16 range, the bias cancels in the normalization.

v3 layout/scheduling notes:
- q/k feature rows are host-permuted so rope's rotate-half partner always
  sits 16 rows away inside the same 32-partition quadrant: the rotate
  becomes one DVE stream_shuffle instead of 4 SBUF-SBUF DMAs. Scores are
  invariant to any per-head feature permutation applied to both q and k.
- The two heads' score matmuls (64-row contractions) are packed into the
  128x128 PE array concurrently via tile_position row groups.
- Score/AV matmuls skip fully-masked leading columns of diagonal blocks;
  only the remaining 128-wide triangle gets a mask multiply (post-exp).
- AV matmuls flow through a cross-slice pipeline (avq): a j-slice's last
  AV groups issue during the next slice's score groups, so the
  score->exp->mask latency never drains the PE at slice boundaries.
  Each head normalizes immediately after its final AV (the norm chain
  hides behind the other head's AVs).
- Out-projection runs two j-slices behind its own slice, spread across
  the score groups (pp-bank evacs and the normalize chain get a full
  slice of slack); batch 1's QKV projection overlaps batch 0's attention.
- x-slice loads prefetch one slice ahead on the SP DMA queue; const
  loads ride the Activation-engine DMA queue so they never delay an x
  prefetch. The exp activation-table set is preloaded during startup.
- Partial outputs are stored fp16, one batched DMA per (batch, j-slice).
- Pitfall (cost a debugging round): custom-DVE ops (reciprocal_approx_*)
  silently corrupt on HW when their input AP sits at base-partition != 0;
  CoreSim does not model this. Keep recip inputs on a partition-0 tile.
"""

import sys

for _p in ("/opt/trn_rl_repo",):
    if _p not in sys.path:
        sys.path.insert(0, _p)

import numpy as np

B, T, D, H = 2, 2048, 1024, 16
DH = D // H  # 64
N_CORES = 8
HPC = H // N_CORES  # heads per core = 2
BT = B * T  # 4096
ROPE_BASE = 10000.0
EXP_BIAS = -4.0

NK = D // 128       # 8 contraction chunks for qkv projection
NS = BT // 512      # 8 token slices
NJ = T // 512       # 4 tq slices per batch
NB = T // 128       # 16 tk blocks per batch

# rope feature permutation within one 64-row head: rows [0:32] hold
# features [0..15, 32..47], rows [32:64] hold [16..31, 48..63] — the
# rotate-half partner of row r is always (r+16)%32 within r's quadrant.
PERM64 = (list(range(0, 16)) + list(range(32, 48))
          + list(range(16, 32)) + list(range(48, 64)))
SHUF_MASK = [(i + 16) % 32 for i in range(32)]

_CACHE = {}


def _host_consts():
    inv_freq = 1.0 / (ROPE_BASE ** (np.arange(0, DH, 2, dtype=np.float32) / DH))
    t = np.arange(T, dtype=np.float32)
    # per permuted row r: feature f = PERM64[r], freq index f%32,
    # sin sign - for f<32 (gets -x2 partner), + for f>=32
    cos64 = np.empty((64, T), dtype=np.float32)
    sin64 = np.empty((64, T), dtype=np.float32)
    for r, f in enumerate(PERM64):
        ang = t * inv_freq[f % 32]
        cos64[r] = np.cos(ang)
        sin64[r] = np.sin(ang) * (-1.0 if f < 32 else 1.0)
    cosb = np.concatenate([cos64, cos64], axis=0).astype(np.float16)
    sinb = np.concatenate([sin64, sin64], axis=0).astype(np.float16)
    # upper-triangle keep-mask for the 128-wide partially-valid column
    # band of each diagonal block: tri[p, c] = 1 if c >= p
    p = np.arange(128)
    tri = (p[None, :] >= p[:, None]).astype(np.float16)
    return cosb, sinb, tri


def _build(debug=False, reps=1, parts="full"):
    """Build + schedule the per-core Bass module (same program on all cores).
    reps>1 repeats the whole body (timing diagnostic: slope between reps
    NEFFs isolates pure body time). parts: timing-bisect variants — "full",
    "noout" (skip out-projection), "noav" (skip AV/norm/outproj too),
    "noatt" (qkv+rope+vprep only). Non-full variants produce garbage
    outputs; timing only."""
    from concourse import bacc
    import concourse.mybir as mybir
    import concourse.tile as tile

    F16 = mybir.dt.float16
    F32 = mybir.dt.float32
    AF = mybir.ActivationFunctionType

    nc = bacc.Bacc("TRN2", target_bir_lowering=False, debug=False,
                   num_devices=N_CORES)

    xt_d = nc.dram_tensor("xt", [D, BT], F16, kind="ExternalInput")
    # weights host-prearranged to [128 partitions, k*128] so the load is
    # contiguous per partition (2KB chunks = full DMA bandwidth)
    wq_d = nc.dram_tensor("wq", [128, D], F16, kind="ExternalInput")
    wk_d = nc.dram_tensor("wk", [128, D], F16, kind="ExternalInput")
    wv_d = nc.dram_tensor("wv", [128, D], F16, kind="ExternalInput")
    wo_d = nc.dram_tensor("wo", [128, D], F16, kind="ExternalInput")
    cos_d = nc.dram_tensor("cosb", [128, T], F16, kind="ExternalInput")
    sin_d = nc.dram_tensor("sinb", [128, T], F16, kind="ExternalInput")
    tri_d = nc.dram_tensor("tri", [128, 128], F16, kind="ExternalInput")
    out_d = nc.dram_tensor("outp", [D, BT], F16, kind="ExternalOutput")
    out_r = out_d[:, :].rearrange("(m p) t -> p m t", p=128)

    with tile.TileContext(nc) as tc:
        with (
            tc.tile_pool(name="consts", bufs=1) as consts,
            tc.tile_pool(name="acts", bufs=1) as acts,
            tc.tile_pool(name="on", bufs=2) as onp,
            tc.tile_pool(name="xt", bufs=3) as xtp,
            tc.tile_pool(name="rope", bufs=4) as rope,
            tc.tile_pool(name="vp", bufs=2) as vpp,
            tc.tile_pool(name="est", bufs=10) as estp,
            tc.tile_pool(name="inv", bufs=2) as invp,
            tc.tile_pool(name="oev", bufs=2) as oevp,
            tc.tile_pool(name="pp_ps", bufs=2, space="PSUM") as pp_ps,
            tc.tile_pool(name="st_ps", bufs=2, space="PSUM") as st_ps,
            tc.tile_pool(name="u_ps", bufs=1, space="PSUM") as u_ps,
        ):
            wq = consts.tile([128, NK, 128], F16)
            wk = consts.tile([128, NK, 128], F16)
            wv = consts.tile([128, NK, 128], F16)
            wo = consts.tile([128, NK, 128], F16)
            cosb = consts.tile([128, T], F16)
            sinb = consts.tile([128, T], F16)
            tri = consts.tile([128, 128], F16)
            # qkv weights first so the first projection matmul can start
            # as early as possible; cos/sin next (first rope), wo/tri later
            nc.sync.dma_start(out=wq, in_=wq_d[:, :].rearrange("p (k f) -> p k f", k=NK))
            ones16 = consts.tile([128, NB], F16)
            nc.vector.memset(ones16, 1.0)
            ebias = consts.tile([128, 1], F32)
            nc.vector.memset(ebias, EXP_BIAS)
            # dummy matmuls fill the initial input-DMA window so the PE's
            # HAM clock-gate is warm (2.4GHz) when the real stream starts
            warm = consts.tile([128, 512], F16)
            nc.vector.memset(warm, 0.0)
            wmp = pp_ps.tile([128, 512], F32, tag="pp", name="warm_ps")
            for _ in range(8):
                nc.tensor.matmul(wmp, warm[:, 0:128], warm,
                                 start=True, stop=True)

            qt = acts.tile([128, BT], F16)  # rows: [h0 d0..63 | h1 d0..63]
            kt = acts.tile([128, BT], F16)
            vt = acts.tile([128, BT], F16)

            xt_r = xt_d[:, :].rearrange("(k p) t -> p k t", p=128)

            rep = 0

            def qkv_load(n):
                xtt = xtp.tile([128, NK, 512], F16, tag="xt",
                               name=f"xtt{n}_r{rep}")
                nc.sync.dma_start(out=xtt, in_=xt_r[:, :, n * 512:(n + 1) * 512])
                return xtt

            def qkv_slice(n, xtt=None):
                """QKV^T projection + rope for 512-token slice n. xtt may be
                a single [128,NK,512] tile or a tuple of equal sub-splits.
                q/k PSUM evacuations run on ACT (keeps DVE free for rope +
                outproj evacs; measured faster on HW than splitting them)."""
                ts = slice(n * 512, (n + 1) * 512)
                cs = slice((n % NJ) * 512, (n % NJ) * 512 + 512)
                on_act = True
                if xtt is None:
                    xtt = qkv_load(n)
                halves = xtt if isinstance(xtt, (list, tuple)) else (xtt,)
                kper = NK // len(halves)
                for w, dst in ((wq, qt), (wk, kt), (wv, vt)):
                    ps = pp_ps.tile([128, 512], F32, tag="pp")
                    for k in range(NK):
                        nc.tensor.matmul(ps, w[:, k, :],
                                         halves[k // kper][:, k % kper, :],
                                         start=(k == 0), stop=(k == NK - 1))
                    if dst is vt:
                        nc.vector.tensor_copy(vt[:, ts], ps)
                    else:
                        raw = rope.tile([128, 512], F16, tag="raw")
                        if on_act:
                            nc.scalar.copy(raw, ps)
                        else:
                            nc.vector.tensor_copy(raw, ps)
                        swp = rope.tile([128, 512], F16, tag="swp")
                        nc.vector.stream_shuffle(swp, raw, SHUF_MASK)
                        nc.vector.tensor_mul(raw, raw, cosb[:, cs])
                        nc.vector.tensor_mul(swp, swp, sinb[:, cs])
                        nc.vector.tensor_add(dst[:, ts], raw, swp)

            def vprep_alloc(b):
                """V' = [V_h | 1] token-major; inner stride 80 elems = 160B
                keeps every transpose dst 32B-aligned (unaligned dsts
                corrupt silently). The constant ones column is written once
                per allocation (early, no deps) rather than per slice."""
                vph = [vpp.tile([128, NB, 80], F16, tag=f"vp{h}",
                                name=f"vp{h}_{b}_r{rep}")
                       for h in range(HPC)]
                for h in range(HPC):
                    nc.vector.tensor_copy(vph[h][:, :, 64], ones16[:, 0:NB])
                return vph

            def vprep_slice(b, vph, s):
                """Transpose one 512-token slice (4 blocks) of V per head."""
                t0 = b * T
                for h in range(HPC):
                    nc.sync.dma_start_transpose(
                        out=vph[h][:, 4 * s:4 * s + 4, 0:64],
                        in_=vt[h * 64:(h + 1) * 64,
                               t0 + s * 512:t0 + (s + 1) * 512])

            def att_jslice(b, j, vph, on, deferred):
                """Attention for q tokens [512j, 512j+512) of batch b, both
                heads. The previous j-slice's out-projection is spread
                across this slice's score groups (hides pp-bank evac
                latency); each head is normalized right after its final AV
                so the norm chain hides behind the other head's AVs."""
                t0 = b * T
                qs0 = t0 + j * 512
                nblk = 4 * j + 4
                ngrp = nblk // 2
                us = [u_ps.tile([65, 512], F32, tag=f"u{h}",
                                name=f"u{h}_{b}_{j}_r{rep}") for h in range(HPC)]

                def norm_head(h):
                    # r = u[64] (denominators); on = u[0:64] / r. u's two
                    # reads (numerator copy + denominator row) come first so
                    # the bank frees early; recip input must sit at
                    # base-partition 0 (custom-DVE op).
                    hp = h * 64
                    rrow = invp.tile([1, 512], F32, tag="rrow")
                    nc.vector.tensor_copy(rrow, us[h][64:65, :])
                    ucop = invp.tile([64, 512], F32, tag="ucop")
                    nc.vector.tensor_copy(ucop, us[h][0:64, :])
                    rinv = invp.tile([1, 512], F32, tag="rinv")
                    nc.vector.reciprocal_approx_fast(rinv, rrow)
                    bci = invp.tile([64, 512], F32, tag="bci")
                    nc.gpsimd.partition_broadcast(bci, rinv)
                    nc.gpsimd.tensor_mul(
                        on[hp:hp + 64, j * 512:(j + 1) * 512],
                        ucop, bci)

                def make_av(h, est, blks):
                    # AV step as a closure on the rep-wide avq pipeline:
                    # entries survive j-slice boundaries, so the next
                    # slice's scores hide this slice's est-chain latency
                    def run():
                        for t2, i, o in blks:
                            nc.tensor.matmul(
                                us[h][:, o:512], vph[h][:, i, 0:65],
                                est[:, t2 * 512 + o:(t2 + 1) * 512],
                                start=(i == 0), stop=(i == nblk - 1))
                        if blks[-1][1] == nblk - 1:
                            norm_head(h)
                    return run

                def run_deferred(g):
                    # run outproj steps two j-slices behind their own slice:
                    # only entries older than the newest NK run here, so the
                    # normalize chain producing `on` has a full j-slice of
                    # slack before the first outproj matmul reads it
                    avail = len(deferred) - NK
                    if avail > 0:
                        for _ in range(-(-avail // (ngrp - g))):
                            deferred.pop(0)()

                for g in range(ngrp):
                    for h in range(HPC):
                        hp = h * 64
                        st = st_ps.tile([128, 1024], F32, tag="st")
                        blks = []
                        for t2 in range(2):
                            i = 2 * g + t2
                            o = max(0, 128 * (i - 4 * j))
                            blks.append((t2, i, o))
                            nc.tensor.matmul(
                                st[:, t2 * 512 + o:(t2 + 1) * 512],
                                kt[hp:hp + 64,
                                   t0 + i * 128: t0 + (i + 1) * 128],
                                qt[hp:hp + 64, qs0 + o: qs0 + 512],
                                start=True, stop=True,
                                tile_position=(hp, 0))
                        est = estp.tile([128, 1024], F16, tag="est")
                        if blks[0][2] == 0 and blks[1][2] == 0:
                            nc.scalar.activation(est, st, AF.Exp,
                                                 scale=float(DH) ** -0.5,
                                                 bias=ebias)
                        else:  # diagonal group: exp only the written bands
                            for t2, i, o in blks:
                                c0, c1 = t2 * 512 + o, (t2 + 1) * 512
                                nc.scalar.activation(est[:, c0:c1],
                                                     st[:, c0:c1], AF.Exp,
                                                     scale=float(DH) ** -0.5,
                                                     bias=ebias)
                        for t2, i, o in blks:
                            if i >= 4 * j:  # diagonal: mask the 128-wide band
                                c0 = t2 * 512 + o
                                nc.gpsimd.tensor_mul(
                                    est[:, c0:c0 + 128], est[:, c0:c0 + 128],
                                    tri)
                        if parts not in ("noav",):
                            avq.append(make_av(h, est, blks))
                        if len(avq) > 3:
                            avq.pop(0)()
                    run_deferred(g)  # spread 2-slice-old outproj over groups
                if b == B - 1 and j == NJ - 1:
                    while avq:  # final drain, then keep PE warm through the
                        avq.pop(0)()  # last normalize + out-projection
                    wst = st_ps.tile([128, 1024], F32, tag="st",
                                     name=f"warm_tail_r{rep}")
                    for _ in range(12):
                        nc.tensor.matmul(wst[:, 0:512], warm[:, 0:128], warm,
                                         start=True, stop=True)

                # out-projection for this j-slice, as per-m steps the next
                # j-slice interleaves with its score groups; one batched
                # store DMA at the end (split in two for the final slice)
                ons = on[:, j * 512:(j + 1) * 512]
                ot = oevp.tile([128, NK, 512], F16, tag="ot",
                               name=f"ot_{b}_{j}_r{rep}")
                split = (b == B - 1 and j == NJ - 1)

                def outproj_step(m):
                    def run():
                        op = pp_ps.tile([128, 512], F32, tag="pp",
                                        name=f"op_{b}_{j}_{m}_r{rep}")
                        nc.tensor.matmul(op, wo[:, m, :], ons,
                                         start=True, stop=True)
                        nc.vector.tensor_copy(ot[:, m, :], op)
                        if split and m == NK // 2 - 1:
                            nc.sync.dma_start(
                                out=out_r[:, 0:NK // 2,
                                          t0 + j * 512: t0 + (j + 1) * 512],
                                in_=ot[:, 0:NK // 2, :])
                        if m == NK - 1:
                            nc.sync.dma_start(
                                out=out_r[:, NK // 2 if split else 0: NK,
                                          t0 + j * 512: t0 + (j + 1) * 512],
                                in_=ot[:, NK // 2 if split else 0: NK, :])
                    return run
                if parts == "full":
                    deferred.extend(outproj_step(m) for m in range(NK))

            # ---- pipelined emission: qkv(b0) | att(b0) x qkv(b1) | att(b1)
            # startup: first x-slice load goes on the DMA queue right after
            # wq, split in two so the first matmuls start after half loads;
            # wk/wv/cos/sin/wo/tri follow behind it
            xt0a = xtp.tile([128, NK // 2, 512], F16, tag="x0a", name="xt0a")
            nc.sync.dma_start(out=xt0a, in_=xt_r[:, 0:NK // 2, 0:512])
            xt0b = xtp.tile([128, NK // 2, 512], F16, tag="x0b", name="xt0b")
            nc.sync.dma_start(out=xt0b, in_=xt_r[:, NK // 2:NK, 0:512])
            xtt0 = (xt0a, xt0b)
            # remaining consts go on the Activation-engine DMA queue so the
            # SP queue carries only x loads + output stores (a const never
            # delays an x prefetch); const DMAs have no waits, so they can't
            # head-of-line block ACT's activations
            nc.scalar.dma_start(out=wk, in_=wk_d[:, :].rearrange("p (k f) -> p k f", k=NK))
            nc.scalar.dma_start(out=wv, in_=wv_d[:, :].rearrange("p (k f) -> p k f", k=NK))
            # slice 1 also split in halves so its first matmuls start early
            xt1a = xtp.tile([128, NK // 2, 512], F16, tag="x1a", name="xt1a")
            nc.sync.dma_start(out=xt1a, in_=xt_r[:, 0:NK // 2, 512:1024])
            xt1b = xtp.tile([128, NK // 2, 512], F16, tag="x1b", name="xt1b")
            nc.sync.dma_start(out=xt1b, in_=xt_r[:, NK // 2:NK, 512:1024])
            xtt1 = (xt1a, xt1b)
            nc.scalar.dma_start(out=cosb, in_=cos_d[:, :])
            nc.scalar.dma_start(out=sinb, in_=sin_d[:, :])
            nc.scalar.dma_start(out=wo, in_=wo_d[:, :].rearrange("p (m f) -> p m f", m=NK))
            nc.scalar.dma_start(out=tri, in_=tri_d[:, :])
            # preload the exp table set while ACT is otherwise idle so the
            # first real exp (on the attention critical path) doesn't pay
            # the ~2.7µs table-load
            wexp = consts.tile([128, 1], F32)
            nc.scalar.activation(wexp, ebias, AF.Exp)
            deferred = []
            avq = []  # cross-slice AV pipeline (see make_av)
            for rep in range(reps):
                if rep > 0:
                    xtt0 = qkv_load(0)
                    xtt1 = qkv_load(1)
                vph0 = vprep_alloc(0)
                qkv_slice(0, xtt0)
                vprep_slice(0, vph0, 0)
                xttn = qkv_load(2)  # prefetch one slice ahead
                qkv_slice(1, xtt1)
                vprep_slice(0, vph0, 1)
                for n in range(2, 4):
                    xtt, xttn = xttn, qkv_load(n + 1)
                    qkv_slice(n, xtt)
                    vprep_slice(0, vph0, n)
                vph1 = vprep_alloc(1)
                on0 = onp.tile([128, T], F16, tag="on", name=f"on0_r{rep}")
                on1 = onp.tile([128, T], F16, tag="on", name=f"on1_r{rep}")
                for j in range(NJ):
                    if parts != "noatt":
                        att_jslice(0, j, vph0, on0, deferred)
                    xtt = xttn
                    if j < NJ - 1:
                        xttn = qkv_load(5 + j)
                    qkv_slice(4 + j, xtt)
                    vprep_slice(1, vph1, j)
                if parts != "noatt":
                    for j in range(NJ):
                        att_jslice(1, j, vph1, on1, deferred)
            while deferred:
                deferred.pop(0)()

    nc.compile()
    return nc


def _get_nc(debug=False):
    key = "ncd" if debug else "nc"
    if key not in _CACHE:
        _CACHE[key] = _build(debug)
    return _CACHE[key]


def _run(nc, in_maps, trace=False):
    from concourse.bass_utils import run_bass_kernel_spmd

    last = None
    for attempt in range(3):
        try:
            return run_bass_kernel_spmd(nc, in_maps,
                                        core_ids=list(range(N_CORES)),
                                        trace=trace)
        except Exception as e:  # transient device faults: retry
            last = e
            if "UNRECOVERABLE" not in str(e) and "UNAVAILABLE" not in str(e):
                raise
    raise last


def _in_maps(x, w_qkv, w_out):
    x = np.asarray(x, dtype=np.float32)
    w_qkv = np.asarray(w_qkv, dtype=np.float32)
    w_out = np.asarray(w_out, dtype=np.float32)

    xt = np.ascontiguousarray(x.reshape(BT, D).T).astype(np.float16)
    cosb, sinb, tri = _host_consts()

    def wprep(w128):  # [D, 128] -> [128, k*128] partition-contiguous
        return np.ascontiguousarray(
            w128.reshape(NK, 128, 128).transpose(1, 0, 2).reshape(128, D)
        ).astype(np.float16)

    in_maps = []
    for c in range(N_CORES):
        h0 = HPC * c
        # q/k columns in rope-permuted order; v/wo in natural order
        pcols = np.concatenate(
            [np.array(PERM64) + (h0 + h) * DH for h in range(HPC)])
        cols = np.arange(h0 * DH, (h0 + HPC) * DH)
        in_maps.append({
            "xt": xt,
            "wq": wprep(w_qkv[:, pcols]),
            "wk": wprep(w_qkv[:, D + pcols]),
            "wv": wprep(w_qkv[:, 2 * D + cols]),
            "wo": np.ascontiguousarray(w_out[cols, :]).astype(np.float16),
            "cosb": cosb,
            "sinb": sinb,
            "tri": tri,
        })
    return in_maps


def kernel(x, w_qkv, w_out, _trace=False, _debug=False):
    in_maps = _in_maps(x, w_qkv, w_out)
    nc = _get_nc(_debug)
    res = _run(nc, in_maps, trace=_trace)
    acc = np.zeros((D, BT), dtype=np.float64)
    for c in range(N_CORES):
        acc += res.results[c]["outp"]
    out = acc.T.astype(np.float32).reshape(B, T, D)
    if _trace:
        return out, res
    return out

